# revision 55
# baseline (speedup 1.0000x reference)
"""Longformer attention Trainium2 kernel (8 NeuronCores, sequence-sharded).

Reference computation (B=1, L=4096, D=1024, H=16, HD=64, WINDOW=15):
  Q/K/V = x @ W{q,k,v}.T ; RoPE on Q,K ; mask = (causal & |i-j|<=7) | (j==0) | (i==0)
  out = softmax(QK^T/8 + mask) @ V @ Wo.T

Sharding: core c owns queries/keys [c*512, (c+1)*512).  Host precomputes the
tiny boundary data (K/V for the 8 positions preceding each shard, corner V
rows, the pos-0 row for the global key/query; all <1 GFLOP total) so every
device-side GEMM runs full-width N=512 chains.  Per head the banded scores
are four block-diagonal [128k x 128q] matmuls + one merged [32x32] corner
matmul + one global-key row, with multiplicative masks after a raw exp.

Scheduling: engines execute their queues in order, so the emission order is
a software pipeline — attention heads are split into score/AV parts with a
one-head lookahead, and the V/K projection chains are interleaved into the
attention phase so the PE always has independent GEMM work while the
exp/mask/normalize chains run on Act/DVE/Pool.  DMA count is kept small
(~35/run) because each dma_start costs ~0.6us of SP sequencer issue time.
The global query row 0 is combined on the host via per-core online-softmax
partials.
"""

import os
import numpy as np
import ml_dtypes
from contextlib import ExitStack

import concourse.bass as bass
import concourse.tile as tile
from concourse import bacc, mybir
from concourse import bass_utils

P = 128
L = 4096
D = 1024
H = 16
HD = 64
NC = 8
LLOC = L // NC          # 512 queries/keys per core
NT = D // P             # 8 channel tiles
NQB = LLOC // P         # 4 query blocks per core
KW = LLOC + 8           # krot columns: [8 boundary keys | 512 owned keys]
BF = mybir.dt.bfloat16
F32 = mybir.dt.float32
NPBF = ml_dtypes.bfloat16

# packed-const column offsets (bf16 [128, CPK])
_CS, _SN, _MB = 0, 512, 1024
_ID, _Q0, _K0 = 1536, 1664, 1680
_KP, _MC = 1696, 1760
_VC, _V0 = 1792, 2832
CPK = 3872

_CACHE = {}


def _build_module(loop_reps=0):
    key = ("nc", loop_reps, os.environ.get("KDBG", "0"), os.environ.get("KWARM", "1"))
    if key in _CACHE:
        return _CACHE[key]
    nc = bacc.Bacc("TRN2", target_bir_lowering=False, debug=False,
                   enable_asserts=False, num_devices=NC)

    def din(name, shape, dt=BF):
        return nc.dram_tensor(name, shape, dt, kind="ExternalInput").ap()

    xT = din("xT", [D, LLOC])               # [ch, 512] bf16, owned positions
    wqT = din("wqT", [D, D])                # head-permuted for RoPE layout
    wkT = din("wkT", [D, D])
    wvT = din("wvT", [D, D])
    woT = din("woT", [D, D])
    cpk = din("cpk", [P, CPK])              # packed constants (see offsets)

    yT = nc.dram_tensor("yT", [NT, P, LLOC], BF, kind="ExternalOutput").ap()
    o0p = nc.dram_tensor("o0p", [2, 512], F32, kind="ExternalOutput").ap()
    lm = nc.dram_tensor("lm", [H, 2], F32, kind="ExternalOutput").ap()
    KDBG = os.environ.get("KDBG", "0") == "1"
    if KDBG:
        dbg = {n: nc.dram_tensor(n, s, BF, kind="ExternalOutput").ap()
               for n, s in [("d_qrot", [P, NT, LLOC]), ("d_krot", [P, NT, KW]),
                            ("d_eS", [P, LLOC]),
                            ("d_eT", [32, 32]), ("d_stage", [HD + 1, LLOC])]}

    with tile.TileContext(nc) as tc, ExitStack() as ctx:
        if loop_reps:
            ctx.enter_context(tc.For_i(0, loop_reps, 1))
        const = ctx.enter_context(tc.tile_pool(name="const", bufs=1))
        big = ctx.enter_context(tc.tile_pool(name="big", bufs=1))
        work = ctx.enter_context(tc.tile_pool(name="work", bufs=3))
        att = ctx.enter_context(tc.tile_pool(name="att", bufs=4))
        row0 = ctx.enter_context(tc.tile_pool(name="row0", bufs=2))
        ps = ctx.enter_context(tc.tile_pool(name="ps", bufs=2, space="PSUM"))
        ps1 = ctx.enter_context(tc.tile_pool(name="ps1", bufs=2, space="PSUM"))
        ps2 = ctx.enter_context(tc.tile_pool(name="ps2", bufs=2, space="PSUM"))
        ps3 = ctx.enter_context(tc.tile_pool(name="ps3", bufs=2, space="PSUM"))

        # ---- PE warmup: keep the HAM clock-gate busy during the loads ----
        if os.environ.get("KWARM", "1") == "1":
            wtiny = work.tile([P, P], BF, tag="wtiny")
            nc.vector.memset(wtiny[:], 0.0)
            for _ in range(12):
                wps = ps3.tile([34, 512], F32, tag="small", name="wps")
                nc.tensor.matmul(wps[0:32, 0:P], lhsT=wtiny[:, 0:32],
                                 rhs=wtiny[:], start=True, stop=True)

        # ---- resident loads: split so Qproj's inputs land first ----
        xTs = const.tile([P, NT, LLOC], BF, tag="xTs")
        xv = xT.rearrange("(t p) d -> p t d", p=P)

        wpool = ctx.enter_context(tc.tile_pool(name="wpool", bufs=3))

        def loadw(ap_dram, name, nsplit=2, defer=None):
            t = wpool.tile([P, NT, D], BF, tag="w", name=name)
            r = ap_dram.rearrange("(t p) d -> p t d", p=P)
            w_ = D // nsplit
            for s in range(nsplit):
                nc.sync.dma_start(t[:, :, s * w_:(s + 1) * w_],
                                  r[:, :, s * w_:(s + 1) * w_])
            return t

        wq_s = wpool.tile([P, NT, D], BF, tag="w", name="wq")
        wqv = wqT.rearrange("(t p) d -> p t d", p=P)
        nc.sync.dma_start(xTs[:, 0:4, :], xv[:, 0:4, :])
        nc.sync.dma_start(wq_s[:, :, 0:256], wqv[:, :, 0:256])
        nc.sync.dma_start(xTs[:, 4:8, :], xv[:, 4:8, :])
        for s in range(1, 4):
            nc.sync.dma_start(wq_s[:, :, s * 256:(s + 1) * 256],
                              wqv[:, :, s * 256:(s + 1) * 256])
        cp = const.tile([P, CPK], BF, tag="cpk")
        nc.sync.dma_start(cp[:], cpk)
        wv_s = loadw(wvT, "wv")
        wk_s = loadw(wkT, "wk")

        cs_s = cp[:, _CS:_CS + LLOC]
        sn_s = cp[:, _SN:_SN + LLOC]
        mB_s = cp[:, _MB:_MB + LLOC]
        id_s = cp[:, _ID:_ID + P]
        q0_s = cp[:, _Q0:_Q0 + H]
        k0_s = cp[:, _K0:_K0 + H]
        mC_s = cp[0:32, _MC:_MC + 32]
        vcorn = cp[0:32, _VC:_VC + H * (HD + 1)].rearrange(
            "p (h d) -> p h d", h=H)

        qrot = big.tile([P, NT, LLOC], BF)
        krot = big.tile([P, NT, KW], BF)
        nc.sync.dma_start(krot[:, :, 0:8], cp[:, _KP:_KP + NT * 8])

        egs = big.tile([P, NT, LLOC], BF)     # exp(global scores), head h at
                                              # partition 64*(h%2), col h//2
        qcorn = big.tile([P, NT, 32], BF)     # gathered corner query cols
        kcorn = big.tile([P, NT, 32], BF)     # gathered corner key cols
        s0 = row0.tile([H, 512], F32, tag="s0")
        EXP = mybir.ActivationFunctionType.Exp

        qtp = ctx.enter_context(tc.tile_pool(name="qtp", bufs=3))

        def proj_group(w_s, rot, coff, tiles, gather_q=False, gather_k=False,
                       copy_eng=None):
            # matmuls + psum copies for the group's tiles, one batched
            # partition swap (32<->32 within each 64-row head half), rope muls
            ng = len(tiles)
            qtbig = qtp.tile([P, 4, LLOC], BF, tag="qtbig")
            swqb = qtp.tile([P, 4, LLOC], BF, tag="swqb")
            for i, t in enumerate(tiles):
                pj = ps.tile([P, 512], F32, tag="sA")
                for k in range(NT):
                    nc.tensor.matmul(pj[:], lhsT=w_s[:, k, t * P:(t + 1) * P],
                                     rhs=xTs[:, k, :],
                                     start=(k == 0), stop=(k == NT - 1))
                if copy_eng == "alt" and i % 2 == 1:
                    nc.vector.tensor_copy(qtbig[:, i, :], pj[:])
                else:
                    nc.scalar.copy(qtbig[:, i, :], pj[:])
            for hb2 in (0, HD):
                nc.sync.dma_start(swqb[hb2:hb2 + 32, 0:ng],
                                  qtbig[hb2 + 32:hb2 + HD, 0:ng])
                nc.sync.dma_start(swqb[hb2 + 32:hb2 + HD, 0:ng],
                                  qtbig[hb2:hb2 + 32, 0:ng])
            ng_ = len(tiles)
            t0 = tiles[0]
            rsl = rot[:, t0:t0 + ng_, coff:coff + LLOC]
            t2 = work.tile([P, 4, LLOC], BF, tag="t2")
            snb = sn_s[:, None, :].to_broadcast((P, ng_, LLOC))
            csb = cs_s[:, None, :].to_broadcast((P, ng_, LLOC))
            nc.vector.tensor_mul(t2[:, 0:ng_], swqb[:, 0:ng_], snb)
            nc.vector.tensor_mul(rsl, qtbig[:, 0:ng_], csb)
            nc.vector.tensor_add(rsl, rsl, t2[:, 0:ng_])
            for i, t in enumerate(tiles):
                if gather_q:
                    nc.vector.tensor_copy(
                        qcorn[:, t].rearrange("p (m c) -> p m c", c=8),
                        qrot[:, t, :].rearrange("p (m c) -> p m c", c=P)[:, :, 0:8])
                if gather_k:
                    nc.vector.tensor_copy(
                        kcorn[:, t].rearrange("p (m c) -> p m c", c=8),
                        krot[:, t, 0:512].rearrange(
                            "p (m c) -> p m c", c=P)[:, :, 0:8])
                    sp = ps3.tile([34, 512], F32, tag="small",
                                  name="sp")[32:34, :]
                    nc.tensor.matmul(sp, lhsT=q0_s[:, 2 * t:2 * t + 2],
                                     rhs=krot[:, t, 8:KW], start=True, stop=True)
                    sc = row0.tile([2, 512], F32, tag="s0c")
                    nc.scalar.copy(sc[:], sp)
                    nc.sync.dma_start(s0[2 * t:2 * t + 2, :], sc[:])

        def sg_block(hps):
            # global-key score rows for the heads of the given qrot tiles
            for hp in hps:
                for h in (2 * hp, 2 * hp + 1):
                    pb = 64 * (h % 2)
                    sg = ps3.tile([34, 512], F32, tag="small",
                                  name="sg")[0:1, :]
                    nc.tensor.matmul(sg, lhsT=k0_s[:, h:h + 1],
                                     rhs=qrot[:, hp, :], start=True, stop=True)
                    nc.scalar.activation(egs[pb:pb + 1, h // 2, :], sg,
                                         EXP, scale=0.125)

        def vproj(cc, js, copy_eng=None):
            for j in js:
                pv = ps.tile([P, 512], F32, tag="sA")
                for k in range(NT):
                    nc.tensor.matmul(
                        pv[:],
                        lhsT=xTs[:, k, j * P:(j + 1) * P],
                        rhs=wv_s[:, k, cc * 512:(cc + 1) * 512],
                        start=(k == 0), stop=(k == NT - 1))
                if copy_eng == "alt" and j % 2 == 1:
                    nc.vector.tensor_copy(
                        v_s[:, j, cc * 8:(cc + 1) * 8, 1:1 + HD],
                        pv[:].rearrange("p (h d) -> p h d", h=8))
                else:
                    nc.scalar.copy(
                        v_s[:, j, cc * 8:(cc + 1) * 8, 1:1 + HD],
                        pv[:].rearrange("p (h d) -> p h d", h=8))

        # ---- attention head, split for software pipelining ----
        stages = big.tile([HD + 1, 8, LLOC], BF, name="stages")
        ot_all = big.tile([P, NT, LLOC], BF)
        att_state = {}

        def score_part(h):
            hb, hp = (h % 2) * HD, h // 2
            Krow = krot[hb:hb + HD, hp]
            Qrow = qrot[hb:hb + HD, hp]
            small = ps3.tile([34, 512], F32, tag="small", name="small")
            # block-diagonal scores: key r=pos qs+r on partitions, query cols
            sS = ps1.tile([P, 512], F32, tag="sB")
            for m in range(NQB):
                nc.tensor.matmul(sS[:, m * P:(m + 1) * P],
                                 lhsT=Krow[:, 8 + m * P:8 + (m + 1) * P],
                                 rhs=Qrow[:, m * P:(m + 1) * P],
                                 start=True, stop=True)
            # merged corner: keys {m*128-8+rr}, queries {m*128+ii}
            sT = small[0:32, 0:32]
            nc.tensor.matmul(sT, lhsT=kcorn[hb:hb + HD, hp],
                             rhs=qcorn[hb:hb + HD, hp], start=True, stop=True)
            # exp + multiplicative masks
            eS = att.tile([P, 512], BF, tag="eS")
            nc.scalar.activation(eS[:], sS[:], EXP, scale=0.125)
            nc.vector.tensor_mul(eS[:], eS[:], mB_s)
            eT = att.tile([32, 32], BF, tag="eT")
            nc.scalar.activation(eT[:], sT, EXP, scale=0.125)
            nc.vector.tensor_mul(eT[:], eT[:], mC_s)
            att_state[h] = (eS, eT)

        def av_part(h):
            eS, eT = att_state.pop(h)
            # O^T_aug accumulation: global key (full-width, opens the psum
            # group) + diag blocks + corner spills
            oa = ps2.tile([HD + 1, 512], F32, tag="oA")
            pb = 64 * (h % 2)
            hsl = slice(_V0 + h * (HD + 1), _V0 + (h + 1) * (HD + 1))
            nc.tensor.matmul(oa[:], lhsT=cp[pb:pb + 1, hsl],
                             rhs=egs[pb:pb + 1, h // 2, :],
                             start=True, stop=False, skip_group_check=True)
            for m in range(NQB):
                nc.tensor.matmul(oa[:, m * P:(m + 1) * P],
                                 lhsT=v_s[:, m, h, :],
                                 rhs=eS[:, m * P:(m + 1) * P],
                                 start=False, stop=False, skip_group_check=True)
            oc = oa[:].rearrange("p (m c) -> p m c", c=P)[:, :, 0:8]
            nc.tensor.matmul(oc, lhsT=vcorn[:, h, :], rhs=eT[:],
                             start=False, stop=True, skip_group_check=True)
            # normalize by denominator (row 0)
            rsc = att.tile([1, 512], F32, tag="rsc")
            nc.vector.reciprocal_approx_fast(rsc[:], oa[0:1, :])
            rb = att.tile([HD + 1, 512], F32, tag="rb")
            nc.gpsimd.partition_broadcast(rb[:], rsc[:])
            nc.vector.tensor_mul(stages[:, h % 8, :], oa[:], rb[:])
            if KDBG and h == 0:
                nc.sync.dma_start(dbg["d_eS"], eS[:])
                nc.sync.dma_start(dbg["d_eT"], eT[:])
                nc.sync.dma_start(dbg["d_stage"], stages[:, 0, :])
            if h in (7, 13, 15):
                sl = {7: slice(0, 8), 13: slice(0, 6), 15: slice(6, 8)}[h]
                tt = {7: slice(0, 4), 13: slice(4, 7), 15: slice(7, 8)}[h]
                sv2 = stages[1:1 + HD, sl, :].rearrange(
                    "p (hp par) c -> p par hp c", par=2)
                nc.sync.dma_start(ot_all[0:HD, tt, :], sv2[:, 0])
                nc.sync.dma_start(ot_all[HD:P, tt, :], sv2[:, 1])

        # ---- emission schedule (software pipeline) ----
        v_s = big.tile([P, NQB, H, HD + 1], BF)
        nc.vector.memset(v_s[:, :, :, 0:1], 1.0)

        proj_group(wq_s, qrot, 0, [0, 1, 2, 3], gather_q=True)
        proj_group(wq_s, qrot, 0, [4, 5, 6, 7], gather_q=True)
        sg_block([0, 1, 2, 3])
        wo_s = loadw(woT, "wo")        # reuses wq's buffer once Qproj is done
        sg_block([4, 5, 6, 7])
        vproj(0, [0, 1, 2, 3])
        proj_group(wk_s, krot, 8, [0, 1], gather_k=True)
        proj_group(wk_s, krot, 8, [2, 3], gather_k=True)
        score_part(0)
        for h in (0, 1, 2):
            score_part(h + 1)
            av_part(h)
        proj_group(wk_s, krot, 8, [4, 5], gather_k=True)
        for h in (3, 4, 5):
            score_part(h + 1)
            av_part(h)
        vproj(1, [0, 1])
        score_part(7)
        av_part(6)
        vproj(1, [2, 3])
        score_part(8)
        av_part(7)
        proj_group(wk_s, krot, 8, [6, 7], gather_k=True)
        score_part(9)
        for h in range(8, 14):
            score_part(h + 2)
            av_part(h)
        av_part(14)
        av_part(15)

        if KDBG:
            nc.sync.dma_start(dbg["d_qrot"], qrot[:])
            nc.sync.dma_start(dbg["d_krot"], krot[:])

        # ---- row-0 softmax prep (Act/DVE; overlaps outproj) ----
        lmx = row0.tile([H, 2], F32, tag="lmx")
        l0 = lmx[:, 0:1]
        nc.vector.memset(lmx[:, 1:2], 0.0)      # m0 = 0 (raw-exp partials)
        e0 = row0.tile([H, 512], BF, tag="e0")
        nc.scalar.activation(e0[:], s0[:], EXP,
                             scale=0.125, accum_out=l0)

        # ---- row-0 weighted-V partials (fills PE while ot DMA lands) ----
        e0t = row0.tile([P, NQB, H], BF, tag="e0t")
        for j in range(NQB):
            tp = ps1.tile([P, H], BF, tag="sB")
            nc.tensor.transpose(tp[:], e0[:, j * P:(j + 1) * P], id_s[0:H, 0:H])
            nc.scalar.copy(e0t[:, j, :], tp[:])
        o0ps = ps.tile([33, 512], F32, tag="sA")
        o0a = o0ps[0:1, :]
        o0b = o0ps[32:33, :]
        for j in range(NQB):
            m2 = row0.tile([P, H, HD], BF, tag="m2")
            nc.vector.tensor_mul(m2[:], v_s[:, j, :, 1:1 + HD],
                                 e0t[:, j, :, None].to_broadcast((P, H, HD)))
            nc.tensor.matmul(o0a[:], lhsT=v_s[:, 0, 0, 0:1],
                             rhs=m2[:, 0:8, :], start=(j == 0), stop=(j == NQB - 1))
            nc.tensor.matmul(o0b[:], lhsT=v_s[:, 0, 0, 0:1],
                             rhs=m2[:, 8:H, :], start=(j == 0), stop=(j == NQB - 1))
        o0sa = row0.tile([1, 512], F32, tag="o0sa")
        o0sb = row0.tile([1, 512], F32, tag="o0sb")
        nc.scalar.copy(o0sa[:], o0a[:])
        nc.scalar.copy(o0sb[:], o0b[:])
        nc.sync.dma_start(o0p[0:1, :], o0sa[:])
        nc.sync.dma_start(o0p[1:2, :], o0sb[:])
        nc.sync.dma_start(lm[:], lmx[:])

        # ---- output projection: y^T = Wo @ O^T ----
        yall = big.tile([P, NT, LLOC], BF, name="yall")
        for t in range(NT):
            fp = ps.tile([P, 512], F32, tag="sA")
            for k in range(NT):
                nc.tensor.matmul(fp[:], lhsT=wo_s[:, k, t * P:(t + 1) * P],
                                 rhs=ot_all[:, k, :],
                                 start=(k == 0), stop=(k == NT - 1))
            nc.scalar.copy(yall[:, t, :], fp[:])
            if t in (3, 5, 7):
                tt = {3: slice(0, 4), 5: slice(4, 6), 7: slice(6, 8)}[t]
                nc.sync.dma_start(yT[tt].rearrange("t p c -> p t c"),
                                  yall[:, tt, :])

    nc.compile()
    _CACHE[key] = nc
    return nc


def _host_inputs(x, Wq, Wk, Wv, Wo, freqs_cos, freqs_sin):
    x2 = np.asarray(x, np.float32).reshape(L, D)
    fc = np.asarray(freqs_cos, np.float32).reshape(L, HD // 2)
    fs = np.asarray(freqs_sin, np.float32).reshape(L, HD // 2)
    bf = lambda a: np.ascontiguousarray(a).astype(NPBF)
    # permute Q/K output channels: within each head, channel (d) -> slot
    # (d%2)*32 + d//2, so RoPE pairs occupy partition blocks [0:32|32:64]
    hperm = np.empty(D, np.int64)
    for h_ in range(H):
        for d_ in range(HD):
            hperm[h_ * HD + (d_ % 2) * 32 + d_ // 2] = h_ * HD + d_
    WqTp = np.asarray(Wq, np.float32).T[:, hperm]      # [D(in), D(perm out)]
    WkTp = np.asarray(Wk, np.float32).T[:, hperm]
    WvT = np.asarray(Wv, np.float32).T
    shared = {
        "wqT": bf(WqTp),
        "wkT": bf(WkTp),
        "wvT": bf(WvT),
        "woT": bf(np.asarray(Wo, np.float32).T),
    }
    # band mask [key r, query col m*128+i]: allowed iff 0 <= i - r <= 7
    rr = np.arange(P)[:, None]
    ii = np.arange(P)[None, :]
    band = ((ii - rr >= 0) & (ii - rr <= 7)).astype(np.float32)
    maskB = np.tile(band, (1, NQB))
    # core 0, block 0: key 0 is the global key — it is added via the global
    # term, so remove it from the band to avoid double counting
    maskB0 = maskB.copy()
    maskB0[0, 0:P] = 0.0
    # corner mask [8m+rr, 8m'+ii]: m==m' and rr >= ii+1
    mq = np.arange(32)[:, None] // 8
    rq = np.arange(32)[:, None] % 8
    mi = np.arange(32)[None, :] // 8
    qi = np.arange(32)[None, :] % 8
    corner = ((mq == mi) & (rq >= qi + 1)).astype(np.float32)
    corner0 = corner * (mq > 0)          # core 0: no keys before position 0

    # pos-0 row data (global key/query), unrotated (RoPE at pos 0 = identity)
    q0 = x2[0] @ np.asarray(Wq, np.float32).T          # [1024]
    k0 = x2[0] @ np.asarray(Wk, np.float32).T
    v0 = x2[0] @ WvT
    q0p, k0p = q0[hperm], k0[hperm]
    q0a = np.zeros((P, H), np.float32)
    k0a = np.zeros((P, H), np.float32)
    for h_ in range(H):
        hb = (h_ % 2) * HD
        q0a[hb:hb + HD, h_] = q0p[h_ * HD:(h_ + 1) * HD]
        k0a[hb:hb + HD, h_] = k0p[h_ * HD:(h_ + 1) * HD]
    v0aug = np.concatenate([np.ones((H, 1), np.float32),
                            v0.reshape(H, HD)], axis=1).reshape(-1)

    # RoPE tables for the owned positions; row layout: [32 a-rows | 32 b-rows]
    # per head half, repeated for both heads of a tile
    f = np.arange(HD) % 32                             # permuted row -> freq
    sgn = np.where(np.arange(HD) < 32, -1.0, 1.0)[:, None]

    def rope_apply(vec_perm, pos):
        # vec_perm [n, D] permuted channels; returns rotated, permuted
        out = np.empty_like(vec_perm)
        for h_ in range(H):
            a = vec_perm[:, h_ * HD:h_ * HD + 32]
            b = vec_perm[:, h_ * HD + 32:h_ * HD + HD]
            c = fc[pos]
            s = fs[pos]
            out[:, h_ * HD:h_ * HD + 32] = a * c - b * s
            out[:, h_ * HD + 32:h_ * HD + HD] = a * s + b * c
        return out

    in_maps = []
    for c in range(NC):
        start = c * LLOC
        pos = np.arange(start, start + LLOC)
        cse = fc[pos][:, f].T                          # [64, 512]
        sne = fs[pos][:, f].T
        cpk = np.zeros((P, CPK), np.float32)
        cpk[:, _CS:_CS + LLOC] = np.concatenate([cse, cse], axis=0)
        cpk[:, _SN:_SN + LLOC] = np.concatenate([sne * sgn, sne * sgn], axis=0)
        cpk[:, _MB:_MB + LLOC] = maskB0 if c == 0 else maskB
        cpk[:, _ID:_ID + P] = np.eye(P, dtype=np.float32)
        cpk[:, _Q0:_Q0 + H] = q0a
        cpk[:, _K0:_K0 + H] = k0a
        cpk[0:32, _MC:_MC + 32] = corner0 if c == 0 else corner
        cpk[0, _V0:_V0 + H * (HD + 1)] = v0aug
        cpk[64, _V0:_V0 + H * (HD + 1)] = v0aug
        # boundary keys: positions start-8..start-1 (zeros for core 0)
        ppos = np.arange(start - 8, start)
        if c > 0:
            kp = rope_apply(x2[ppos] @ WkTp, ppos)     # [8, D] rotated, perm
            cpk[:, _KP:_KP + NT * 8] = kp.T.reshape(NT, P, 8).transpose(
                1, 0, 2).reshape(P, NT * 8)
        # corner V rows: positions start + m*128 - 8 + rr (m 0..3, rr 0..7)
        vc = np.zeros((32, H, HD + 1), np.float32)
        for m in range(NQB):
            cp_ = np.arange(start + m * P - 8, start + m * P)
            if c == 0 and m == 0:
                continue
            vv = x2[cp_] @ WvT
            vc[8 * m:8 * m + 8, :, 0] = 1.0
            vc[8 * m:8 * m + 8, :, 1:] = vv.reshape(8, H, HD)
        cpk[0:32, _VC:_VC + H * (HD + 1)] = vc.reshape(32, -1)
        im = dict(shared)
        im["xT"] = bf(x2[pos].T)
        im["cpk"] = bf(cpk)
        in_maps.append(im)
    return in_maps


def _assemble(results, Wo):
    y = np.empty((L, D), np.float32)
    for c in range(NC):
        yt = results[c]["yT"].astype(np.float32).reshape(D, LLOC)
        y[c * LLOC:(c + 1) * LLOC] = yt.T
    # combine row-0 online-softmax partials
    m0 = np.stack([results[c]["lm"][:, 1] for c in range(NC)])         # [NC, H]
    l0 = np.stack([results[c]["lm"][:, 0] for c in range(NC)])
    o0 = np.stack([results[c]["o0p"].reshape(2, 8, HD).reshape(H, HD)
                   for c in range(NC)])                                # [NC, H, 64]
    mstar = m0.max(axis=0)
    alpha = np.exp(0.125 * (m0 - mstar[None]))                         # [NC, H]
    num = (alpha[:, :, None] * o0).sum(axis=0)                         # [H, 64]
    den = (alpha * l0).sum(axis=0)                                     # [H]
    row0 = (num / den[:, None]).reshape(D)
    y[0] = row0 @ np.asarray(Wo, np.float32).T
    return y.reshape(1, L, D)


def kernel(x, Wq, Wk, Wv, Wo, freqs_cos, freqs_sin):
    nc = _build_module()
    in_maps = _host_inputs(x, Wq, Wk, Wv, Wo, freqs_cos, freqs_sin)
    res = bass_utils.run_bass_kernel_spmd(nc, in_maps, core_ids=list(range(NC)))
    return _assemble(res.results, Wo)
